# revision 1
# baseline (speedup 1.0000x reference)
"""Trainium2 Bass kernel for multi-head attention (B=2, S=2048, D=2048, 16 heads).

Sharding: 8 cores = 2 batch groups (data parallel) x 4 tensor-parallel ranks.
Each core handles one batch element and 4 heads (wqkv rows / wo cols sharded by
head). Partial output projections are summed with a per-query-chunk
ReduceScatter over each 4-core group; the host reassembles the full output.

Layout tricks:
- All matmul inputs are pre-transposed host-side so the device only ever does
  natural [K-on-partition] matmuls: xt = x^T, wqkvt = wqkv_shard^T, wot = wo_shard^T.
- Head-dim of q/k is permuted (even indices then odd) host-side, so RoPE pairs
  land on partition j / 64+j in the [hd, s] layout; scores are invariant to the
  permutation since q and k share it.
- q is pre-scaled by HD^-0.5 by scaling the rotary tables on device.
"""

import sys
import numpy as np
import ml_dtypes

sys.path.insert(0, "/opt/trn_rl_repo")

B, S, D = 2, 2048, 2048
NH, HD = 16, 128
TP = 4            # tensor-parallel ranks per batch group
HL = NH // TP     # heads per core = 4
DL = HL * HD      # local out-proj contraction = 512
NDT = D // 128    # 16 d-tiles
NQT = S // 128    # 16 q-tiles
NQC = 4           # 512-row query chunks
SM_SCALE = float(HD) ** -0.5
GROUPS = [[0, 1, 2, 3], [4, 5, 6, 7]]

_cache = {}


def _build_graph():
    import concourse.bass as bass
    import concourse.mybir as mybir
    import concourse.tile as tile
    from concourse import bacc
    from concourse.masks import make_identity

    f32 = mybir.dt.float32
    bf16 = mybir.dt.bfloat16
    AF = mybir.ActivationFunctionType
    OP = mybir.AluOpType

    nc = bacc.Bacc("TRN2", target_bir_lowering=False, debug=False, num_devices=8)

    xt_ext = nc.declare_dram_parameter("xt", [D, S], bf16, isOutput=False)
    wqkvt_ext = nc.declare_dram_parameter("wqkvt", [D, 3 * DL], bf16, isOutput=False)
    wot_ext = nc.declare_dram_parameter("wot", [DL, D], bf16, isOutput=False)
    cost_ext = nc.declare_dram_parameter("cost", [HD // 2, S], f32, isOutput=False)
    sint_ext = nc.declare_dram_parameter("sint", [HD // 2, S], f32, isOutput=False)
    maskd_ext = nc.declare_dram_parameter("maskd", [S, 128], f32, isOutput=False)
    out_ext = nc.declare_dram_parameter("out", [NQC * 128, D], bf16, isOutput=True)

    with tile.TileContext(nc) as tc:
        # pools that live across both phases
        with tc.tile_pool(name="pers", bufs=1) as pers, \
             tc.tile_pool(name="dram", bufs=1, space="DRAM") as dram:
            # persistent tensors used by attention
            qk_bf = [pers.tile([128, S], bf16, tag=f"qk{i}", name=f"qk{i}")
                     for i in range(2 * HL)]            # 4 q heads then 4 k heads, [hd, s]
            v_bf = [pers.tile([128, DL], bf16, tag=f"v{i}", name=f"v{i}")
                    for i in range(NQT)]                # [s-tile, 4*hd']
            ident = pers.tile([128, 128], bf16, tag="ident", name="ident")
            make_identity(nc, ident[:])
            mask_sb = [pers.tile([128, 128], f32, tag=f"mk{t}", name=f"mk{t}")
                       for t in range(NQT)]
            wo_bf = [pers.tile([128, D], bf16, tag=f"wo{h}", name=f"wo{h}")
                     for h in range(HL)]
            for t in range(NQT):
                nc.sync.dma_start(out=mask_sb[t][:],
                                  in_=maskd_ext[t * 128:(t + 1) * 128, :])
                nc.vector.tensor_scalar_max(out=mask_sb[t][:], in0=mask_sb[t][:],
                                            scalar1=-1e30)
            for h in range(HL):
                nc.sync.dma_start(out=wo_bf[h][:],
                                  in_=wot_ext[h * 128:(h + 1) * 128, :])

            # ---------------- Phase A: QKV projection + RoPE ----------------
            with tc.tile_pool(name="early", bufs=1) as early, \
                 tc.tile_pool(name="ldw", bufs=4) as ldw, \
                 tc.tile_pool(name="rope", bufs=1) as rope_pool:
                xt_bf = [early.tile([128, S], bf16, tag=f"xt{i}", name=f"xt{i}")
                         for i in range(NDT)]
                vt_sb = [early.tile([128, S], bf16, tag=f"vt{i}", name=f"vt{i}")
                         for i in range(HL)]
                cq = early.tile([HD // 2, S], f32, tag="cq", name="cq")
                sq = early.tile([HD // 2, S], f32, tag="sq", name="sq")
                ck = early.tile([HD // 2, S], f32, tag="ck", name="ck")
                sk = early.tile([HD // 2, S], f32, tag="sk", name="sk")

                nc.sync.dma_start(out=ck[:], in_=cost_ext[:])
                nc.sync.dma_start(out=sk[:], in_=sint_ext[:])
                nc.vector.tensor_scalar_mul(out=cq[:], in0=ck[:], scalar1=SM_SCALE)
                nc.vector.tensor_scalar_mul(out=sq[:], in0=sk[:], scalar1=SM_SCALE)

                for dt in range(NDT):
                    nc.sync.dma_start(out=xt_bf[dt][:, 0:S // 2],
                                      in_=xt_ext[dt * 128:(dt + 1) * 128, 0:S // 2])
                    nc.sync.dma_start(out=xt_bf[dt][:, S // 2:],
                                      in_=xt_ext[dt * 128:(dt + 1) * 128, S // 2:])

                # Q/K in [e, s] layout: lhsT = wqkvt tile, rhs = xt tile
                with tc.tile_pool(name="psQK", bufs=1, space="PSUM") as psQK:
                  for et in range(3 * HL):
                      wq_t = [ldw.tile([128, 128], bf16, tag=f"wq{dt % 4}",
                                       name=f"wq_{et}_{dt}") for dt in range(NDT)]
                      for dt in range(NDT):
                          nc.sync.dma_start(
                              out=wq_t[dt][:],
                              in_=wqkvt_ext[dt * 128:(dt + 1) * 128,
                                            et * 128:(et + 1) * 128])
                      ps_qk = psQK.tile([128, S], f32, tag=f"ps_qk{et % 2}",
                                       name=f"ps_qk{et}", bufs=1)
                      for dt in range(NDT):
                          for sc in range(4):
                              nc.tensor.matmul(
                                  ps_qk[:, sc * 512:(sc + 1) * 512],
                                  wq_t[dt][:],
                                  xt_bf[dt][:, sc * 512:(sc + 1) * 512],
                                  start=(dt == 0), stop=(dt == NDT - 1))
                      if et >= 2 * HL:
                          # v head: stash [hd', s] for later PE transpose
                          nc.scalar.copy(vt_sb[et - 2 * HL][:], ps_qk[:])
                          continue
                      # RoPE on [hd, s]: rows 0:64 = even pairs (r), 64:128 = odd (i)
                      c = cq if et < HL else ck
                      s_ = sq if et < HL else sk
                      r = ps_qk[0:64, :]
                      i_ = ps_qk[64:128, :]
                      t1 = rope_pool.tile([64, S], f32, tag="t1", name=f"t1_{et}")
                      t2 = rope_pool.tile([64, S], f32, tag="t2", name=f"t2_{et}")
                      nc.vector.tensor_tensor(out=t1[:], in0=r, in1=c[:], op=OP.mult)
                      nc.vector.tensor_tensor(out=t2[:], in0=i_, in1=s_[:], op=OP.mult)
                      nc.vector.tensor_tensor(out=qk_bf[et][0:64, :], in0=t1[:],
                                              in1=t2[:], op=OP.subtract)
                      nc.vector.tensor_tensor(out=t1[:], in0=r, in1=s_[:], op=OP.mult)
                      nc.vector.tensor_tensor(out=t2[:], in0=i_, in1=c[:], op=OP.mult)
                      nc.vector.tensor_tensor(out=qk_bf[et][64:128, :], in0=t1[:],
                                              in1=t2[:], op=OP.add)

                # transpose vT [hd', s] -> V natural [s-tile, hd'] blocks
                with tc.tile_pool(name="psVT", bufs=2, space="PSUM") as psVT:
                    for vt in range(HL):
                        for st in range(NQT):
                            ps_vt = psVT.tile([128, 128], bf16, tag="ps_vt",
                                              name=f"ps_vt_{vt}_{st}")
                            nc.tensor.transpose(
                                ps_vt[:], vt_sb[vt][:, st * 128:(st + 1) * 128],
                                ident[:])
                            eng = nc.vector if st % 2 == 0 else nc.scalar
                            if st % 2 == 0:
                                nc.vector.tensor_copy(
                                    v_bf[st][:, vt * 128:(vt + 1) * 128], ps_vt[:])
                            else:
                                nc.scalar.copy(
                                    v_bf[st][:, vt * 128:(vt + 1) * 128], ps_vt[:])

            # ---------------- Phase B: attention + out-proj + RS ----------------
            # Heads are processed in interleaved pairs so two independent
            # softmax chains keep the PE fed; PT transposes are batched 4 per
            # PSUM bank and copied out with one strided op.
            with tc.tile_pool(name="late", bufs=1) as late, \
                 tc.tile_pool(name="att", bufs=4) as att, \
                 tc.tile_pool(name="psB", bufs=2, space="PSUM") as psB:
                o2_bf = [late.tile([128, S], bf16, tag=f"o2{h}", name=f"o2{h}")
                         for h in range(HL)]
                pt_all = [late.tile([128, NQT * 512], bf16, tag=f"pta{j}",
                                    name=f"pta{j}") for j in range(2)]
                pending_out = []

                def softmax_pt(qc, h, sub):
                    j = h % 2
                    qt = qc * 4 + sub
                    nk = qt + 1
                    kw_total = nk * 128
                    nchunks = (kw_total + 511) // 512
                    exp_sb = att.tile([128, S], f32, tag="exp",
                                      name=f"exp_{qc}_{h}_{sub}", bufs=4)
                    rs_parts = att.tile([128, 4], f32, tag="rsp",
                                        name=f"rsp_{qc}_{h}_{sub}")
                    for ckk in range(nchunks):
                        kw = min(512, kw_total - ckk * 512)
                        ps_sc = psB.tile([128, 512], f32, tag="ps_sc",
                                         name=f"ps_sc_{qc}_{h}_{sub}_{ckk}")
                        nc.tensor.matmul(
                            ps_sc[:, 0:kw],
                            qk_bf[h][:, qt * 128:(qt + 1) * 128],
                            qk_bf[HL + h][:, ckk * 512:ckk * 512 + kw],
                            start=True, stop=True)
                        if ckk == nchunks - 1:
                            nc.vector.tensor_tensor(
                                out=ps_sc[:, kw - 128:kw],
                                in0=ps_sc[:, kw - 128:kw],
                                in1=mask_sb[qt][:], op=OP.add)
                        nc.scalar.activation(
                            exp_sb[:, ckk * 512:ckk * 512 + kw],
                            ps_sc[:, 0:kw], AF.Exp,
                            accum_out=rs_parts[:, ckk:ckk + 1])
                    recip = att.tile([128, 1], f32, tag="recip",
                                     name=f"recip_{qc}_{h}_{sub}")
                    if nchunks > 1:
                        rsum = att.tile([128, 1], f32, tag="rsum",
                                        name=f"rsum_{qc}_{h}_{sub}")
                        nc.vector.tensor_reduce(
                            out=rsum[:], in_=rs_parts[:, 0:nchunks],
                            axis=mybir.AxisListType.X, op=OP.add)
                        nc.vector.reciprocal(recip[:], rsum[:])
                    else:
                        nc.vector.reciprocal(recip[:], rs_parts[:, 0:1])
                    p_bf = att.tile([128, S], bf16, tag="pbf",
                                    name=f"pbf_{qc}_{h}_{sub}", bufs=3)
                    nc.vector.tensor_scalar(
                        out=p_bf[:, 0:kw_total], in0=exp_sb[:, 0:kw_total],
                        scalar1=recip[:], scalar2=None, op0=OP.mult)
                    # zero-fill diagonal-block PT region once per (h, qc)
                    if sub == 0:
                        for kt in range(qc * 4, qc * 4 + 4):
                            nc.vector.memset(
                                pt_all[j][:, kt * 512:(kt + 1) * 512], 0.0)
                    # PE transposes in groups of 8 (one full PSUM bank)
                    # -> one strided copy out
                    pt_view = pt_all[j][:].rearrange("p (k c) -> p k c", c=512)
                    for g in range((nk + 7) // 8):
                        gs = min(8, nk - g * 8)
                        ps_pt8 = psB.tile([128, 1024], bf16, tag="ps_pt",
                                          name=f"ps_pt_{qc}_{h}_{sub}_{g}")
                        for kk in range(gs):
                            kt = g * 8 + kk
                            nc.tensor.transpose(
                                ps_pt8[:, kk * 128:(kk + 1) * 128],
                                p_bf[:, kt * 128:(kt + 1) * 128], ident[:])
                        dst = pt_view[:, g * 8:g * 8 + gs,
                                      sub * 128:(sub + 1) * 128]
                        srcv = ps_pt8[:, 0:gs * 128].rearrange(
                            "p (k c) -> p k c", c=128)
                        if g % 2 == 0:
                            nc.vector.tensor_copy(dst, srcv)
                        else:
                            nc.scalar.copy(dst, srcv)

                def pv(qc, h):
                    j = h % 2
                    nkc = qc * 4 + 4
                    ps_o2 = psB.tile([128, 512], f32, tag="ps_o2",
                                     name=f"ps_o2_{qc}_{h}")
                    for kt in range(nkc):
                        nc.tensor.matmul(
                            ps_o2[:],
                            v_bf[kt][:, h * 128:(h + 1) * 128],
                            pt_all[j][:, kt * 512:(kt + 1) * 512],
                            start=(kt == 0), stop=(kt == nkc - 1))
                    nc.scalar.copy(o2_bf[h][:, qc * 512:(qc + 1) * 512], ps_o2[:])

                for qc in range(NQC):
                    for hp in range(2):
                        for sub in range(4):
                            for h in (2 * hp, 2 * hp + 1):
                                softmax_pt(qc, h, sub)
                        for h in (2 * hp, 2 * hp + 1):
                            pv(qc, h)

                    # out-projection for this query chunk; the last chunk's
                    # ReduceScatter is split in half to shrink the kernel tail
                    bnc = dram.tile([512, D], bf16, tag=f"bnc{qc}", name=f"bnc{qc}")
                    halves = [(0, 2), (2, 4)] if qc == NQC - 1 else [(0, 4)]
                    for (st_lo, st_hi) in halves:
                        for st_l in range(st_lo, st_hi):
                            st = qc * 4 + st_l
                            for ec in range(4):
                                ps_pr = psB.tile([128, 512], f32, tag="ps_pr",
                                                 name=f"ps_pr_{qc}_{st_l}_{ec}")
                                for h in range(HL):
                                    nc.tensor.matmul(
                                        ps_pr[:],
                                        o2_bf[h][:, st * 128:(st + 1) * 128],
                                        wo_bf[h][:, ec * 512:(ec + 1) * 512],
                                        start=(h == 0), stop=(h == HL - 1))
                                fin_sb = att.tile([128, 512], bf16,
                                                  tag="fin",
                                                  name=f"fin_{qc}_{st_l}_{ec}",
                                                  bufs=16)
                                nc.vector.tensor_copy(fin_sb[:], ps_pr[:])
                                nc.sync.dma_start(
                                    out=bnc[st_l * 128:(st_l + 1) * 128,
                                            ec * 512:(ec + 1) * 512],
                                    in_=fin_sb[:])
                        nrow = (st_hi - st_lo) * 128
                        rso = dram.tile([nrow // 4, D], bf16,
                                        tag=f"rso{qc}_{st_lo}",
                                        name=f"rso{qc}_{st_lo}")
                        nc.gpsimd.collective_compute(
                            "ReduceScatter", OP.add,
                            replica_groups=GROUPS,
                            ins=[bnc[st_lo * 128:st_hi * 128, :].opt()],
                            outs=[rso.opt()])
                        out_row = qc * 128 + st_lo * 32
                        pending_out.append((out_row, nrow // 4, rso))

                for (orow, onrow, orso) in pending_out:
                    nc.sync.dma_start(out=out_ext[orow:orow + onrow, :],
                                      in_=orso[:])
    nc.finalize()
    return nc


def _prep_inputs(x, freqs_cos, freqs_sin, mask, wqkv, wo):
    bf = ml_dtypes.bfloat16
    perm = np.concatenate([np.arange(0, HD, 2), np.arange(1, HD, 2)])
    mask2d = np.ascontiguousarray(np.asarray(mask, np.float32).reshape(S, S))
    maskd = np.concatenate(
        [mask2d[t * 128:(t + 1) * 128, t * 128:(t + 1) * 128] for t in range(NQT)],
        axis=0)
    cost = np.ascontiguousarray(np.asarray(freqs_cos, np.float32).T)
    sint = np.ascontiguousarray(np.asarray(freqs_sin, np.float32).T)
    wqkv = np.asarray(wqkv, np.float32)
    wo = np.asarray(wo, np.float32)
    x = np.asarray(x, np.float32)

    in_maps = []
    for c in range(8):
        b, r = divmod(c, TP)
        heads = range(r * HL, (r + 1) * HL)
        rows = []
        for sec in range(2):  # q then k, head-dim permuted
            for h in heads:
                blk = wqkv[sec * D + h * HD: sec * D + (h + 1) * HD]
                rows.append(blk[perm])
        for h in heads:       # v, natural order
            rows.append(wqkv[2 * D + h * HD: 2 * D + (h + 1) * HD])
        wqkv_shard = np.concatenate(rows, axis=0)           # [1536, 2048]
        wqkvt = np.ascontiguousarray(wqkv_shard.T).astype(bf)
        wo_shard = np.concatenate(
            [wo[:, h * HD:(h + 1) * HD] for h in heads], axis=1)  # [2048, 512]
        wot = np.ascontiguousarray(wo_shard.T).astype(bf)
        xt = np.ascontiguousarray(x[b].T).astype(bf)
        in_maps.append({
            "xt": xt, "wqkvt": wqkvt, "wot": wot,
            "cost": cost, "sint": sint, "maskd": maskd,
        })
    return in_maps


def kernel(x, freqs_cos, freqs_sin, mask, wqkv, wo, input_pos=None,
           _want_res=False, _trace=False, _tmpdir=None):
    from concourse.bass_utils import run_bass_kernel_spmd

    if "nc" not in _cache:
        _cache["nc"] = _build_graph()
    nc = _cache["nc"]

    in_maps = _prep_inputs(x, freqs_cos, freqs_sin, mask, wqkv, wo)
    kw = {}
    if _trace:
        kw = dict(trace=True, tmpdir=_tmpdir)
    res = run_bass_kernel_spmd(nc, in_maps, list(range(8)), **kw)

    y = np.empty((B, S, D), np.float32)
    for c in range(8):
        b, r = divmod(c, TP)
        oc = np.asarray(res.results[c]["out"], np.float32)
        for qc in range(NQC - 1):
            y[b, qc * 512 + r * 128: qc * 512 + (r + 1) * 128, :] = \
                oc[qc * 128:(qc + 1) * 128]
        base = (NQC - 1) * 512
        y[b, base + r * 64: base + (r + 1) * 64, :] = oc[384:448]
        y[b, base + 256 + r * 64: base + 256 + (r + 1) * 64, :] = oc[448:512]
    if _want_res:
        return y, res
    return y



# revision 11
# speedup vs baseline: 1.0622x; 1.0622x over previous
"""Trainium2 Bass kernel for multi-head attention (B=2, S=2048, D=2048, 16 heads).

Sharding: 8 cores = 2 batch groups (data parallel) x 4 tensor-parallel ranks.
Each core computes QKV + attention for its 4 heads over its batch element.
Per 512-row query chunk each core computes its partial out-projection and the
four partials are summed with a ReduceScatter over the 4-core group (each core
keeps one 128-row query subtile).  The last-processed chunk's RS is split
384+128 rows so the kernel tail is only a small collective.

Layout:
- All device matmuls contract over the partition dim.  Host pre-transposes:
  xt = x^T, per-head q/k weights as [d, hd] blocks, wv as [d, vcols],
  woT = wo^T.
- Q/K are produced in [hd, s] layout (RoPE pairs permuted even|odd so the
  rotation acts on partition halves); V is produced directly in natural
  [s, hd] layout (stationary = xt tile), so no PE transposes anywhere.
- Scores are computed transposed [k, q]:  exp tiles feed PV directly
  (O^T accumulates in PSUM) and the softmax denominator comes from a
  ones-vector matmul; normalization multiplies O^T by a partition-broadcast
  reciprocal.  Softmax scale is folded into the Exp activation.
"""

import sys
import numpy as np
import ml_dtypes

sys.path.insert(0, "/opt/trn_rl_repo")

B, S, D = 2, 2048, 2048
NH, HD = 16, 128
TP = 4            # tensor-parallel ranks per batch group
HL = NH // TP     # heads per core = 4
NDT = D // 128    # 16 d-tiles
NSC = 4           # 512-col s chunks
NQT = S // 128    # 16
NQC = 4           # 512-row query chunks
SM_SCALE = float(HD) ** -0.5
GROUPS = [[0, 1, 2, 3], [4, 5, 6, 7]]
CHUNK_ORDER = [2, 3, 1, 0]

_cache = {}


def _build_graph():
    import concourse.mybir as mybir
    import concourse.tile as tile
    from concourse import bacc

    f32 = mybir.dt.float32
    bf16 = mybir.dt.bfloat16
    AF = mybir.ActivationFunctionType
    OP = mybir.AluOpType

    nc = bacc.Bacc("TRN2", target_bir_lowering=False, debug=False, num_devices=8)

    xt_ext = nc.declare_dram_parameter("xt", [D, S], bf16, isOutput=False)
    wqk_ext = nc.declare_dram_parameter("wqk", [2 * HL * 128, NDT * 128], bf16,
                                        isOutput=False)
    wv_ext = nc.declare_dram_parameter("wv", [D, HL * HD], bf16, isOutput=False)
    c2_ext = nc.declare_dram_parameter("c2", [128, S], f32, isOutput=False)
    s2_ext = nc.declare_dram_parameter("s2", [128, S], f32, isOutput=False)
    maskT_ext = nc.declare_dram_parameter("maskT", [512, 512], f32, isOutput=False)
    wot_ext = nc.declare_dram_parameter("wot", [HL * HD, D], bf16, isOutput=False)
    out_ext = nc.declare_dram_parameter("out", [NQC * 128, D], bf16, isOutput=True)

    with tile.TileContext(nc) as tc:
        with tc.tile_pool(name="pers", bufs=1) as pers, \
             tc.tile_pool(name="dram", bufs=1, space="DRAM") as dram:
            qk_bf = [pers.tile([128, S], bf16, tag=f"qk{i}", name=f"qk{i}")
                     for i in range(2 * HL)]            # 0-3 q heads, 4-7 k heads
            v_bf = [pers.tile([128, HL * HD], bf16, tag=f"v{t}", name=f"v{t}")
                    for t in range(NQT)]                # [s-tile, 4*hd]
            ones_col = pers.tile([128, 1], bf16, tag="ones", name="ones")
            nc.vector.memset(ones_col[:], 1.0)

            # ---------------- Phase A: QKV projection + RoPE ----------------
            with tc.tile_pool(name="pha", bufs=1) as pha, \
                 tc.tile_pool(name="rope", bufs=2) as ropep, \
                 tc.tile_pool(name="psA", bufs=3, space="PSUM") as psA:
                wq_sb = [pha.tile([128, NDT * 128], bf16, tag=f"wq{et}",
                                  name=f"wq{et}") for et in range(2 * HL)]
                wv_sb = [pha.tile([128, HL * HD], bf16, tag=f"wv{dt}",
                                  name=f"wv{dt}") for dt in range(NDT)]
                xt_sb = [pha.tile([128, S], bf16, tag=f"xt{dt}", name=f"xt{dt}")
                         for dt in range(NDT)]
                c2_sb = pha.tile([128, S], f32, tag="c2", name="c2")
                s2_sb = pha.tile([128, S], f32, tag="s2", name="s2")

                def dma_xt_chunk(sc):
                    for dt in range(NDT):
                        nc.sync.dma_start(
                            out=xt_sb[dt][:, sc * 512:(sc + 1) * 512],
                            in_=xt_ext[dt * 128:(dt + 1) * 128,
                                       sc * 512:(sc + 1) * 512])

                def dma_tab_chunk(sc):
                    cl = slice(sc * 512, (sc + 1) * 512)
                    nc.sync.dma_start(out=c2_sb[:, cl], in_=c2_ext[:, cl])
                    nc.sync.dma_start(out=s2_sb[:, cl], in_=s2_ext[:, cl])

                # DMA issue order tuned so compute starts after ~2.5 MB
                nc.sync.dma_start(out=wq_sb[0][:], in_=wqk_ext[0:128, :])
                dma_xt_chunk(0)
                dma_tab_chunk(0)
                for et in range(1, 2 * HL):
                    nc.sync.dma_start(out=wq_sb[et][:],
                                      in_=wqk_ext[et * 128:(et + 1) * 128, :])
                for dt in range(NDT):
                    nc.sync.dma_start(out=wv_sb[dt][:],
                                      in_=wv_ext[dt * 128:(dt + 1) * 128, :])
                for sc in range(1, NSC):
                    dma_xt_chunk(sc)
                    dma_tab_chunk(sc)

                for sc in range(NSC):
                    cl = slice(sc * 512, (sc + 1) * 512)
                    for et in range(2 * HL):
                        ps = psA.tile([128, 512], f32, tag="psA",
                                      name=f"psA_{sc}_{et}")
                        for dt in range(NDT):
                            nc.tensor.matmul(
                                ps[:], wq_sb[et][:, dt * 128:(dt + 1) * 128],
                                xt_sb[dt][:, cl],
                                start=(dt == 0), stop=(dt == NDT - 1))
                        # u = [r*c; i*c]; w = [-i*s; r*s] (s2n = [-sin; sin],
                        # cross-partition reads stay on the PSUM operand);
                        # qk = u + w = [r*c - i*s; i*c + r*s]
                        u = ropep.tile([128, 512], f32, tag="t1",
                                       name=f"t1_{sc}_{et}")
                        w = ropep.tile([128, 512], f32, tag="t2",
                                       name=f"t2_{sc}_{et}")
                        nc.vector.tensor_tensor(out=u[:], in0=ps[:],
                                                in1=c2_sb[:, cl], op=OP.mult)
                        nc.vector.tensor_tensor(out=w[0:64, :],
                                                in0=ps[64:128, :],
                                                in1=s2_sb[0:64, cl],
                                                op=OP.mult)
                        nc.vector.tensor_tensor(out=w[64:128, :],
                                                in0=ps[0:64, :],
                                                in1=s2_sb[64:128, cl],
                                                op=OP.mult)
                        nc.vector.tensor_tensor(out=qk_bf[et][:, cl],
                                                in0=u[:], in1=w[:], op=OP.add)
                    for stl in range(4):
                        st = sc * 4 + stl
                        psv = psA.tile([128, 512], f32, tag="psA",
                                       name=f"psV_{st}")
                        for dt in range(NDT):
                            nc.tensor.matmul(
                                psv[:], xt_sb[dt][:, st * 128:(st + 1) * 128],
                                wv_sb[dt][:],
                                start=(dt == 0), stop=(dt == NDT - 1))
                        nc.scalar.copy(v_bf[st][:], psv[:])

            # -------- Phase B: attention + AllToAll + local out-proj --------
            with tc.tile_pool(name="phb", bufs=1) as phb, \
                 tc.tile_pool(name="att", bufs=4) as attp, \
                 tc.tile_pool(name="psS", bufs=2, space="PSUM") as psS, \
                 tc.tile_pool(name="psPV", bufs=2, space="PSUM") as psPV, \
                 tc.tile_pool(name="psD", bufs=2, space="PSUM") as psD, \
                 tc.tile_pool(name="psPR", bufs=2, space="PSUM") as psPR:
                maskT_sb = [phb.tile([128, 512], f32, tag=f"mk{j}",
                                     name=f"mk{j}") for j in range(4)]
                wot_sb = [phb.tile([128, D], bf16, tag=f"wo{h}", name=f"wo{h}")
                          for h in range(HL)]
                for j in range(4):
                    nc.sync.dma_start(out=maskT_sb[j][:],
                                      in_=maskT_ext[j * 128:(j + 1) * 128, :])
                for h in range(HL):
                    nc.sync.dma_start(out=wot_sb[h][:],
                                      in_=wot_ext[h * 128:(h + 1) * 128, :])

                o2p_tiles = {}
                pending_out = []

                def attention(qc, hp):
                    qcl = slice(qc * 512, (qc + 1) * 512)
                    nkt = qc * 4 + 4
                    o2p = attp.tile([128, 1024], bf16, tag="o2p",
                                    name=f"o2p_{qc}_{hp}", bufs=3)
                    o2p_tiles[(qc, hp)] = o2p
                    ps_pv = [psPV.tile([128, 512], f32, tag="pv",
                                       name=f"pv_{qc}_{hp}_{i}")
                             for i in range(2)]
                    ps_d = [psD.tile([1, 512], f32, tag="d",
                                     name=f"d_{qc}_{hp}_{i}") for i in range(2)]
                    for kt in range(nkt):
                        for i in range(2):
                            h = 2 * hp + i
                            ps_s = psS.tile([128, 512], f32, tag="s",
                                            name=f"s_{qc}_{hp}_{kt}_{i}")
                            nc.tensor.matmul(
                                ps_s[:], qk_bf[HL + h][:, kt * 128:(kt + 1) * 128],
                                qk_bf[h][:, qcl], start=True, stop=True)
                            if kt >= qc * 4:
                                nc.vector.tensor_tensor(
                                    out=ps_s[:], in0=ps_s[:],
                                    in1=maskT_sb[kt - qc * 4][:], op=OP.add)
                            e_sb = attp.tile([128, 512], bf16, tag="e",
                                             name=f"e_{qc}_{hp}_{kt}_{i}",
                                             bufs=4)
                            nc.scalar.activation(e_sb[:], ps_s[:], AF.Exp,
                                                 scale=SM_SCALE)
                            nc.tensor.matmul(ps_d[i][:], ones_col[:], e_sb[:],
                                             start=(kt == 0),
                                             stop=(kt == nkt - 1))
                            nc.tensor.matmul(
                                ps_pv[i][:], v_bf[kt][:, h * 128:(h + 1) * 128],
                                e_sb[:], start=(kt == 0), stop=(kt == nkt - 1))
                    for i in range(2):
                        d_sb = attp.tile([1, 512], f32, tag="dsb",
                                         name=f"dsb_{qc}_{hp}_{i}")
                        nc.scalar.copy(d_sb[:], ps_d[i][:])
                        db = attp.tile([128, 512], f32, tag="db",
                                       name=f"db_{qc}_{hp}_{i}", bufs=2)
                        nc.gpsimd.partition_broadcast(db[:], d_sb[:])
                        rb = attp.tile([128, 512], f32, tag="rb",
                                       name=f"rb_{qc}_{hp}_{i}", bufs=2)
                        nc.vector.reciprocal(rb[:], db[:])
                        nc.vector.tensor_tensor(
                            out=o2p[:, i * 512:(i + 1) * 512],
                            in0=ps_pv[i][:], in1=rb[:], op=OP.mult)

                def outproj(qc, st_lo, st_hi, out_row):
                    # partial Y for subtiles [st_lo, st_hi) of chunk qc,
                    # ReduceScatter over the 4-core group
                    bnc = dram.tile([(st_hi - st_lo) * 128, D], bf16,
                                    tag=f"bnc{qc}_{st_lo}",
                                    name=f"bnc{qc}_{st_lo}")
                    for stl in range(st_lo, st_hi):
                        for ec in range(4):
                            ps = psPR.tile([128, 512], f32, tag="pr",
                                           name=f"pr_{qc}_{stl}_{ec}")
                            for h in range(HL):
                                lhsT = o2p_tiles[(qc, h // 2)][
                                    :, (h % 2) * 512 + stl * 128:
                                       (h % 2) * 512 + (stl + 1) * 128]
                                nc.tensor.matmul(
                                    ps[:], lhsT,
                                    wot_sb[h][:, ec * 512:(ec + 1) * 512],
                                    start=(h == 0), stop=(h == HL - 1))
                            fin = attp.tile([128, 512], bf16, tag="fin",
                                            name=f"fin_{qc}_{stl}_{ec}",
                                            bufs=8)
                            nc.vector.tensor_copy(fin[:], ps[:])
                            nc.sync.dma_start(
                                out=bnc[(stl - st_lo) * 128:
                                        (stl - st_lo + 1) * 128,
                                        ec * 512:(ec + 1) * 512],
                                in_=fin[:])
                    nrow = (st_hi - st_lo) * 32
                    rso = dram.tile([nrow, D], bf16, tag=f"rso{qc}_{st_lo}",
                                    name=f"rso{qc}_{st_lo}")
                    nc.gpsimd.collective_compute(
                        "ReduceScatter", OP.add, replica_groups=GROUPS,
                        ins=[bnc[:].opt()], outs=[rso.opt()])
                    pending_out.append((out_row, nrow, rso))

                last = CHUNK_ORDER[-1]
                for qc in CHUNK_ORDER:
                    for hp in range(2):
                        attention(qc, hp)
                    if qc == last:
                        outproj(qc, 0, 3, qc * 128)
                        outproj(qc, 3, 4, qc * 128 + 96)
                    else:
                        outproj(qc, 0, 4, qc * 128)

                for (orow, onrow, orso) in pending_out:
                    nc.sync.dma_start(out=out_ext[orow:orow + onrow, :],
                                      in_=orso[:])
    nc.finalize()
    return nc


def _prep_inputs(x, freqs_cos, freqs_sin, mask, wqkv, wo):
    bf = ml_dtypes.bfloat16
    perm = np.concatenate([np.arange(0, HD, 2), np.arange(1, HD, 2)])
    mask2d = np.asarray(mask, np.float32).reshape(S, S)
    maskT = np.ascontiguousarray(np.concatenate(
        [np.maximum(mask2d[0:512, j * 128:(j + 1) * 128].T, -1e30)
         for j in range(4)], axis=0)).astype(np.float32)
    cosT = np.asarray(freqs_cos, np.float32).T   # [64, S]
    sinT = np.asarray(freqs_sin, np.float32).T
    c2 = np.ascontiguousarray(np.concatenate([cosT, cosT], axis=0))
    s2 = np.ascontiguousarray(np.concatenate([-sinT, sinT], axis=0))
    wqkv = np.asarray(wqkv, np.float32)
    wo = np.asarray(wo, np.float32)
    x = np.asarray(x, np.float32)

    in_maps = []
    for c in range(8):
        b, r = divmod(c, TP)
        heads = list(range(r * HL, (r + 1) * HL))
        # q/k weights: per (sec, head) block in SBUF layout [128 p=d%128,
        # (dt c)=hd], i.e. transpose of blk[c, dt*128+p]
        rows = []
        for sec in range(2):
            for h in heads:
                blk = wqkv[sec * D + h * HD: sec * D + (h + 1) * HD][perm]
                b3 = blk.reshape(HD, NDT, 128)          # [hd, dt, p]
                rows.append(np.transpose(b3, (2, 1, 0)).reshape(128, -1))
        wqk = np.ascontiguousarray(np.concatenate(rows, axis=0)).astype(bf)
        wv = np.ascontiguousarray(np.concatenate(
            [wqkv[2 * D + h * HD: 2 * D + (h + 1) * HD].T for h in heads],
            axis=1)).astype(bf)                          # [2048, 512]
        wo_shard = np.concatenate(
            [wo[:, h * HD:(h + 1) * HD] for h in heads], axis=1)  # [2048, 512]
        wot = np.ascontiguousarray(wo_shard.T).astype(bf)         # [512, 2048]
        xt = np.ascontiguousarray(x[b].T).astype(bf)
        in_maps.append({"xt": xt, "wqk": wqk, "wv": wv, "c2": c2, "s2": s2,
                        "maskT": maskT, "wot": wot})
    return in_maps


def kernel(x, freqs_cos, freqs_sin, mask, wqkv, wo, input_pos=None,
           _want_res=False, _trace=False, _tmpdir=None):
    from concourse.bass_utils import run_bass_kernel_spmd

    if "nc" not in _cache:
        _cache["nc"] = _build_graph()
    nc = _cache["nc"]

    in_maps = _prep_inputs(x, freqs_cos, freqs_sin, mask, wqkv, wo)
    kw = {}
    if _trace:
        kw = dict(trace=True, tmpdir=_tmpdir)
    res = run_bass_kernel_spmd(nc, in_maps, list(range(8)), **kw)

    y = np.empty((B, S, D), np.float32)
    for c in range(8):
        b, r = divmod(c, TP)
        oc = np.asarray(res.results[c]["out"], np.float32)
        for qc in range(1, NQC):
            qt = 4 * qc + r
            y[b, qt * 128:(qt + 1) * 128, :] = oc[qc * 128:(qc + 1) * 128]
        # chunk 0 was reduce-scattered as 384+128 rows
        y[b, r * 96:(r + 1) * 96, :] = oc[0:96]
        y[b, 384 + r * 32:384 + (r + 1) * 32, :] = oc[96:128]
    if _want_res:
        return y, res
    return y


# revision 16
# speedup vs baseline: 1.1176x; 1.0522x over previous
"""Trainium2 Bass kernel for multi-head attention (B=2, S=2048, D=2048, 16 heads).

Sharding: 8 cores = 2 batch groups (data parallel) x 4 tensor-parallel ranks.
Each core computes QKV + attention for its 4 heads over its batch element.
Per 512-row query chunk each core computes its partial out-projection and the
four partials are summed with a ReduceScatter over the 4-core group (each core
keeps one 128-row query subtile).  The last-processed chunk's RS is split
384+128 rows so the kernel tail is only a small collective.

Layout:
- All device matmuls contract over the partition dim.  Host pre-transposes:
  xt = x^T, per-head q/k weights as [d, hd] blocks, wv as [d, vcols],
  woT = wo^T.
- Q/K are produced in [hd, s] layout (RoPE pairs permuted even|odd so the
  rotation acts on partition halves); V is produced directly in natural
  [s, hd] layout (stationary = xt tile), so no PE transposes anywhere.
- Scores are computed transposed [k, q]:  exp tiles feed PV directly
  (O^T accumulates in PSUM) and the softmax denominator comes from a
  ones-vector matmul; normalization multiplies O^T by a partition-broadcast
  reciprocal.  Softmax scale is folded into the Exp activation.
"""

import sys
import numpy as np
import ml_dtypes

sys.path.insert(0, "/opt/trn_rl_repo")

B, S, D = 2, 2048, 2048
NH, HD = 16, 128
TP = 4            # tensor-parallel ranks per batch group
HL = NH // TP     # heads per core = 4
NDT = D // 128    # 16 d-tiles
NSC = 4           # 512-col s chunks
NQT = S // 128    # 16
NQC = 4           # 512-row query chunks
SM_SCALE = float(HD) ** -0.5
GROUPS = [[0, 1, 2, 3], [4, 5, 6, 7]]
CHUNK_ORDER = [2, 3, 1, 0]

_cache = {}


def _build_graph():
    import concourse.mybir as mybir
    import concourse.tile as tile
    from concourse import bacc

    f32 = mybir.dt.float32
    bf16 = mybir.dt.bfloat16
    AF = mybir.ActivationFunctionType
    OP = mybir.AluOpType

    nc = bacc.Bacc("TRN2", target_bir_lowering=False, debug=False, num_devices=8)

    xt_ext = nc.declare_dram_parameter("xt", [D, S], bf16, isOutput=False)
    wqk_ext = nc.declare_dram_parameter("wqk", [2 * HL * 128, NDT * 128], bf16,
                                        isOutput=False)
    wv_ext = nc.declare_dram_parameter("wv", [D, HL * HD], bf16, isOutput=False)
    c2_ext = nc.declare_dram_parameter("c2", [128, S], f32, isOutput=False)
    s2_ext = nc.declare_dram_parameter("s2", [128, S], f32, isOutput=False)
    maskT_ext = nc.declare_dram_parameter("maskT", [512, 512], f32, isOutput=False)
    wot_ext = nc.declare_dram_parameter("wot", [HL * HD, D], bf16, isOutput=False)
    out_ext = nc.declare_dram_parameter("out", [NQC * 128, D], bf16, isOutput=True)

    with tile.TileContext(nc) as tc:
        with tc.tile_pool(name="pers", bufs=1) as pers, \
             tc.tile_pool(name="dram", bufs=1, space="DRAM") as dram:
            qk_bf = [pers.tile([128, S], bf16, tag=f"qk{i}", name=f"qk{i}")
                     for i in range(2 * HL)]            # 0-3 q heads, 4-7 k heads
            v_bf = [pers.tile([128, HL * HD], bf16, tag=f"v{t}", name=f"v{t}")
                    for t in range(NQT)]                # [s-tile, 4*hd]
            ones_col = pers.tile([128, 1], bf16, tag="ones", name="ones")
            nc.vector.memset(ones_col[:], 1.0)

            # ---------------- Phase A: QKV projection + RoPE ----------------
            with tc.tile_pool(name="pha", bufs=1) as pha, \
                 tc.tile_pool(name="rope", bufs=2) as ropep, \
                 tc.tile_pool(name="psA", bufs=3, space="PSUM") as psA:
                wq_sb = [pha.tile([128, NDT * 128], bf16, tag=f"wq{et}",
                                  name=f"wq{et}") for et in range(2 * HL)]
                wv_sb = [pha.tile([128, HL * HD], bf16, tag=f"wv{dt}",
                                  name=f"wv{dt}") for dt in range(NDT)]
                xt_sb = [pha.tile([128, S], bf16, tag=f"xt{dt}", name=f"xt{dt}")
                         for dt in range(NDT)]
                c2_sb = pha.tile([128, S], f32, tag="c2", name="c2")
                s2_sb = pha.tile([128, S], f32, tag="s2", name="s2")

                def dma_xt_chunk(sc):
                    for dt in range(NDT):
                        nc.sync.dma_start(
                            out=xt_sb[dt][:, sc * 512:(sc + 1) * 512],
                            in_=xt_ext[dt * 128:(dt + 1) * 128,
                                       sc * 512:(sc + 1) * 512])

                def dma_tab_chunk(sc):
                    cl = slice(sc * 512, (sc + 1) * 512)
                    nc.sync.dma_start(out=c2_sb[:, cl], in_=c2_ext[:, cl])
                    nc.sync.dma_start(out=s2_sb[:, cl], in_=s2_ext[:, cl])

                # DMA issue order tuned so compute starts after ~2.5 MB
                nc.sync.dma_start(out=wq_sb[0][:], in_=wqk_ext[0:128, :])
                dma_xt_chunk(0)
                dma_tab_chunk(0)
                for et in range(1, 2 * HL):
                    nc.sync.dma_start(out=wq_sb[et][:],
                                      in_=wqk_ext[et * 128:(et + 1) * 128, :])
                for dt in range(NDT):
                    nc.sync.dma_start(out=wv_sb[dt][:],
                                      in_=wv_ext[dt * 128:(dt + 1) * 128, :])
                for sc in range(1, NSC):
                    dma_xt_chunk(sc)
                    dma_tab_chunk(sc)

                for sc in range(NSC):
                    cl = slice(sc * 512, (sc + 1) * 512)
                    for et in range(2 * HL):
                        ps = psA.tile([128, 512], f32, tag="psA",
                                      name=f"psA_{sc}_{et}")
                        for dt in range(NDT):
                            nc.tensor.matmul(
                                ps[:], wq_sb[et][:, dt * 128:(dt + 1) * 128],
                                xt_sb[dt][:, cl],
                                start=(dt == 0), stop=(dt == NDT - 1))
                        # u = [r*c; i*c]; w = [-i*s; r*s] (s2n = [-sin; sin],
                        # cross-partition reads stay on the PSUM operand);
                        # qk = u + w = [r*c - i*s; i*c + r*s]
                        u = ropep.tile([128, 512], f32, tag="t1",
                                       name=f"t1_{sc}_{et}")
                        w = ropep.tile([128, 512], f32, tag="t2",
                                       name=f"t2_{sc}_{et}")
                        nc.vector.tensor_tensor(out=u[:], in0=ps[:],
                                                in1=c2_sb[:, cl], op=OP.mult)
                        nc.vector.tensor_tensor(out=w[0:64, :],
                                                in0=ps[64:128, :],
                                                in1=s2_sb[0:64, cl],
                                                op=OP.mult)
                        nc.vector.tensor_tensor(out=w[64:128, :],
                                                in0=ps[0:64, :],
                                                in1=s2_sb[64:128, cl],
                                                op=OP.mult)
                        nc.vector.tensor_tensor(out=qk_bf[et][:, cl],
                                                in0=u[:], in1=w[:], op=OP.add)
                    for stl in range(4):
                        st = sc * 4 + stl
                        psv = psA.tile([128, 512], f32, tag="psA",
                                       name=f"psV_{st}")
                        for dt in range(NDT):
                            nc.tensor.matmul(
                                psv[:], xt_sb[dt][:, st * 128:(st + 1) * 128],
                                wv_sb[dt][:],
                                start=(dt == 0), stop=(dt == NDT - 1))
                        nc.scalar.copy(v_bf[st][:], psv[:])

            # -------- Phase B: attention + AllToAll + local out-proj --------
            with tc.tile_pool(name="phb", bufs=1) as phb, \
                 tc.tile_pool(name="att", bufs=4) as attp, \
                 tc.tile_pool(name="psS", bufs=2, space="PSUM") as psS, \
                 tc.tile_pool(name="psPV", bufs=2, space="PSUM") as psPV, \
                 tc.tile_pool(name="psD", bufs=2, space="PSUM") as psD, \
                 tc.tile_pool(name="psPR", bufs=2, space="PSUM") as psPR:
                maskT_sb = [phb.tile([128, 512], f32, tag=f"mk{j}",
                                     name=f"mk{j}") for j in range(4)]
                wot_sb = [phb.tile([128, D], bf16, tag=f"wo{h}", name=f"wo{h}")
                          for h in range(HL)]
                for j in range(4):
                    nc.sync.dma_start(out=maskT_sb[j][:],
                                      in_=maskT_ext[j * 128:(j + 1) * 128, :])
                for h in range(HL):
                    nc.sync.dma_start(out=wot_sb[h][:],
                                      in_=wot_ext[h * 128:(h + 1) * 128, :])

                o2p_tiles = {}
                pending_out = []

                def attention(qc, hp):
                    qcl = slice(qc * 512, (qc + 1) * 512)
                    nkt = qc * 4 + 4
                    o2p = attp.tile([128, 1024], bf16, tag="o2p",
                                    name=f"o2p_{qc}_{hp}", bufs=3)
                    o2p_tiles[(qc, hp)] = o2p
                    ps_pv = [psPV.tile([128, 512], f32, tag="pv",
                                       name=f"pv_{qc}_{hp}_{i}")
                             for i in range(2)]
                    ps_d = [psD.tile([1, 512], f32, tag="d",
                                     name=f"d_{qc}_{hp}_{i}") for i in range(2)]
                    for kt in range(nkt):
                        for i in range(2):
                            h = 2 * hp + i
                            ps_s = psS.tile([128, 512], f32, tag="s",
                                            name=f"s_{qc}_{hp}_{kt}_{i}")
                            nc.tensor.matmul(
                                ps_s[:], qk_bf[HL + h][:, kt * 128:(kt + 1) * 128],
                                qk_bf[h][:, qcl], start=True, stop=True)
                            if kt >= qc * 4:
                                nc.vector.tensor_tensor(
                                    out=ps_s[:], in0=ps_s[:],
                                    in1=maskT_sb[kt - qc * 4][:], op=OP.add)
                            e_sb = attp.tile([128, 512], bf16, tag="e",
                                             name=f"e_{qc}_{hp}_{kt}_{i}",
                                             bufs=4)
                            nc.scalar.activation(e_sb[:], ps_s[:], AF.Exp,
                                                 scale=SM_SCALE)
                            nc.tensor.matmul(ps_d[i][:], ones_col[:], e_sb[:],
                                             start=(kt == 0),
                                             stop=(kt == nkt - 1))
                            nc.tensor.matmul(
                                ps_pv[i][:], v_bf[kt][:, h * 128:(h + 1) * 128],
                                e_sb[:], start=(kt == 0), stop=(kt == nkt - 1))
                    for i in range(2):
                        d_sb = attp.tile([1, 512], f32, tag="dsb",
                                         name=f"dsb_{qc}_{hp}_{i}")
                        nc.scalar.copy(d_sb[:], ps_d[i][:])
                        db = attp.tile([128, 512], f32, tag="db",
                                       name=f"db_{qc}_{hp}_{i}", bufs=2)
                        nc.gpsimd.partition_broadcast(db[:], d_sb[:])
                        rb = attp.tile([128, 512], f32, tag="rb",
                                       name=f"rb_{qc}_{hp}_{i}", bufs=2)
                        nc.vector.reciprocal_approx_fast(rb[:], db[:])
                        nc.vector.tensor_tensor(
                            out=o2p[:, i * 512:(i + 1) * 512],
                            in0=ps_pv[i][:], in1=rb[:], op=OP.mult)

                pending_rs = []

                def flush_rs():
                    # RS triggers are deferred so they don't block the
                    # per-head partition_broadcasts queued behind them on
                    # the gpsimd engine while the trigger waits for the
                    # previous chunk's fin DMAs.
                    for (bnc, rso) in pending_rs:
                        nc.gpsimd.collective_compute(
                            "ReduceScatter", OP.add, replica_groups=GROUPS,
                            ins=[bnc[:].opt()], outs=[rso.opt()])
                    pending_rs.clear()

                def outproj(qc, st_lo, st_hi, out_row, defer=True):
                    # partial Y for subtiles [st_lo, st_hi) of chunk qc,
                    # ReduceScatter over the 4-core group
                    bnc = dram.tile([(st_hi - st_lo) * 128, D], bf16,
                                    tag=f"bnc{qc}_{st_lo}",
                                    name=f"bnc{qc}_{st_lo}")
                    for stl in range(st_lo, st_hi):
                        for ec in range(4):
                            ps = psPR.tile([128, 512], f32, tag="pr",
                                           name=f"pr_{qc}_{stl}_{ec}")
                            for h in range(HL):
                                lhsT = o2p_tiles[(qc, h // 2)][
                                    :, (h % 2) * 512 + stl * 128:
                                       (h % 2) * 512 + (stl + 1) * 128]
                                nc.tensor.matmul(
                                    ps[:], lhsT,
                                    wot_sb[h][:, ec * 512:(ec + 1) * 512],
                                    start=(h == 0), stop=(h == HL - 1))
                            fin = attp.tile([128, 512], bf16, tag="fin",
                                            name=f"fin_{qc}_{stl}_{ec}",
                                            bufs=8)
                            nc.vector.tensor_copy(fin[:], ps[:])
                            nc.sync.dma_start(
                                out=bnc[(stl - st_lo) * 128:
                                        (stl - st_lo + 1) * 128,
                                        ec * 512:(ec + 1) * 512],
                                in_=fin[:])
                    nrow = (st_hi - st_lo) * 32
                    rso = dram.tile([nrow, D], bf16, tag=f"rso{qc}_{st_lo}",
                                    name=f"rso{qc}_{st_lo}")
                    if defer:
                        pending_rs.append((bnc, rso))
                    else:
                        nc.gpsimd.collective_compute(
                            "ReduceScatter", OP.add, replica_groups=GROUPS,
                            ins=[bnc[:].opt()], outs=[rso.opt()])
                    pending_out.append((out_row, nrow, rso))

                last = CHUNK_ORDER[-1]
                for qc in CHUNK_ORDER:
                    attention(qc, 0)
                    flush_rs()
                    attention(qc, 1)
                    if qc == last:
                        outproj(qc, 0, 3, qc * 128, defer=False)
                        outproj(qc, 3, 4, qc * 128 + 96, defer=False)
                    else:
                        outproj(qc, 0, 4, qc * 128)

                for (orow, onrow, orso) in pending_out:
                    nc.sync.dma_start(out=out_ext[orow:orow + onrow, :],
                                      in_=orso[:])
    nc.finalize()
    return nc


def _prep_inputs(x, freqs_cos, freqs_sin, mask, wqkv, wo):
    bf = ml_dtypes.bfloat16
    perm = np.concatenate([np.arange(0, HD, 2), np.arange(1, HD, 2)])
    mask2d = np.asarray(mask, np.float32).reshape(S, S)
    maskT = np.ascontiguousarray(np.concatenate(
        [np.maximum(mask2d[0:512, j * 128:(j + 1) * 128].T, -1e30)
         for j in range(4)], axis=0)).astype(np.float32)
    cosT = np.asarray(freqs_cos, np.float32).T   # [64, S]
    sinT = np.asarray(freqs_sin, np.float32).T
    c2 = np.ascontiguousarray(np.concatenate([cosT, cosT], axis=0))
    s2 = np.ascontiguousarray(np.concatenate([-sinT, sinT], axis=0))
    wqkv = np.asarray(wqkv, np.float32)
    wo = np.asarray(wo, np.float32)
    x = np.asarray(x, np.float32)

    in_maps = []
    for c in range(8):
        b, r = divmod(c, TP)
        heads = list(range(r * HL, (r + 1) * HL))
        # q/k weights: per (sec, head) block in SBUF layout [128 p=d%128,
        # (dt c)=hd], i.e. transpose of blk[c, dt*128+p]
        rows = []
        for sec in range(2):
            for h in heads:
                blk = wqkv[sec * D + h * HD: sec * D + (h + 1) * HD][perm]
                b3 = blk.reshape(HD, NDT, 128)          # [hd, dt, p]
                rows.append(np.transpose(b3, (2, 1, 0)).reshape(128, -1))
        wqk = np.ascontiguousarray(np.concatenate(rows, axis=0)).astype(bf)
        wv = np.ascontiguousarray(np.concatenate(
            [wqkv[2 * D + h * HD: 2 * D + (h + 1) * HD].T for h in heads],
            axis=1)).astype(bf)                          # [2048, 512]
        wo_shard = np.concatenate(
            [wo[:, h * HD:(h + 1) * HD] for h in heads], axis=1)  # [2048, 512]
        wot = np.ascontiguousarray(wo_shard.T).astype(bf)         # [512, 2048]
        xt = np.ascontiguousarray(x[b].T).astype(bf)
        in_maps.append({"xt": xt, "wqk": wqk, "wv": wv, "c2": c2, "s2": s2,
                        "maskT": maskT, "wot": wot})
    return in_maps


def kernel(x, freqs_cos, freqs_sin, mask, wqkv, wo, input_pos=None,
           _want_res=False, _trace=False, _tmpdir=None):
    from concourse.bass_utils import run_bass_kernel_spmd

    if "nc" not in _cache:
        _cache["nc"] = _build_graph()
    nc = _cache["nc"]

    in_maps = _prep_inputs(x, freqs_cos, freqs_sin, mask, wqkv, wo)
    kw = {}
    if _trace:
        kw = dict(trace=True, tmpdir=_tmpdir)
    res = run_bass_kernel_spmd(nc, in_maps, list(range(8)), **kw)

    y = np.empty((B, S, D), np.float32)
    for c in range(8):
        b, r = divmod(c, TP)
        oc = np.asarray(res.results[c]["out"], np.float32)
        for qc in range(1, NQC):
            qt = 4 * qc + r
            y[b, qt * 128:(qt + 1) * 128, :] = oc[qc * 128:(qc + 1) * 128]
        # chunk 0 was reduce-scattered as 384+128 rows
        y[b, r * 96:(r + 1) * 96, :] = oc[0:96]
        y[b, 384 + r * 32:384 + (r + 1) * 32, :] = oc[96:128]
    if _want_res:
        return y, res
    return y


# revision 27
# speedup vs baseline: 1.2047x; 1.0779x over previous
"""Trainium2 Bass kernel for multi-head attention (B=2, S=2048, D=2048, 16 heads).

Sharding: 8 cores = 2 batch groups (data parallel) x 4 tensor-parallel ranks.
Each core computes QKV + attention for its 4 heads over its batch element.
Per 512-row query chunk the cores exchange their (normalized) attention
outputs O^T with an 8-way AllToAll (one per head-pair half), then each core
contracts the full 2048-dim O rows of the query subtile it owns against the
full wo^T.  This moves ~2x fewer collective bytes than reduce-scattering
partial Y and moves the exchange before the out-projection, shrinking the
kernel tail.  The A2A must span all 8 cores (mesh needs >4), so each core
writes its blocks into both batch-groups' destination slots and picks the
correct source half with rank-conditional DMAs (cc_rank).

Layout:
- All device matmuls contract over the partition dim.  Host pre-transposes:
  xt = x^T, per-head q/k weights as [d, hd] blocks, wv as [d, vcols],
  woT = wo^T.
- Q/K are produced in [hd, s] layout (RoPE pairs permuted even|odd so the
  rotation acts on partition halves); V is produced directly in natural
  [s, hd] layout (stationary = xt tile), so no PE transposes anywhere.
- Scores are computed transposed [k, q]:  exp tiles feed PV directly
  (O^T accumulates in PSUM) and the softmax denominator comes from a
  ones-vector matmul; normalization multiplies O^T by a partition-broadcast
  reciprocal.  Softmax scale is folded into the Exp activation.
"""

import sys
import numpy as np
import ml_dtypes

sys.path.insert(0, "/opt/trn_rl_repo")

B, S, D = 2, 2048, 2048
NH, HD = 16, 128
TP = 4            # tensor-parallel ranks per batch group
HL = NH // TP     # heads per core = 4
NDT = D // 128    # 16 d-tiles
NSC = 4           # 512-col s chunks
NQT = S // 128    # 16
NQC = 4           # 512-row query chunks
SM_SCALE = float(HD) ** -0.5
GROUPS8 = [[0, 1, 2, 3, 4, 5, 6, 7]]
CHUNK_ORDER = [2, 3, 1, 0]

_cache = {}


def _build_graph():
    import concourse.mybir as mybir
    import concourse.tile as tile
    from concourse import bacc

    f32 = mybir.dt.float32
    bf16 = mybir.dt.bfloat16
    AF = mybir.ActivationFunctionType
    OP = mybir.AluOpType

    nc = bacc.Bacc("TRN2", target_bir_lowering=False, debug=False, num_devices=8)

    xt_ext = nc.declare_dram_parameter("xt", [D, S], bf16, isOutput=False)
    wqk_ext = nc.declare_dram_parameter("wqk", [2 * HL * 128, NDT * 128], bf16,
                                        isOutput=False)
    wv_ext = nc.declare_dram_parameter("wv", [D, HL * HD], bf16, isOutput=False)
    c2_ext = nc.declare_dram_parameter("c2", [128, S], f32, isOutput=False)
    s2_ext = nc.declare_dram_parameter("s2", [128, S], f32, isOutput=False)
    maskT_ext = nc.declare_dram_parameter("maskT", [512, 512], f32, isOutput=False)
    woT_ext = nc.declare_dram_parameter("woT", [D, D], bf16, isOutput=False)
    out_ext = nc.declare_dram_parameter("out", [NQC * 128, D], bf16, isOutput=True)

    with tile.TileContext(nc) as tc:
        with tc.tile_pool(name="pers", bufs=1) as pers, \
             tc.tile_pool(name="dram", bufs=1, space="DRAM") as dram:
            qk_bf = [pers.tile([128, S], bf16, tag=f"qk{i}", name=f"qk{i}")
                     for i in range(2 * HL)]            # 0-3 q heads, 4-7 k heads
            v_bf = [pers.tile([128, HL * HD], bf16, tag=f"v{t}", name=f"v{t}")
                    for t in range(NQT)]                # [s-tile, 4*hd]
            ones_col = pers.tile([128, 1], bf16, tag="ones", name="ones")
            nc.vector.memset(ones_col[:], 1.0)

            # ---------------- Phase A: QKV projection + RoPE ----------------
            with tc.tile_pool(name="pha", bufs=1) as pha, \
                 tc.tile_pool(name="rope", bufs=2) as ropep, \
                 tc.tile_pool(name="psA", bufs=3, space="PSUM") as psA:
                wq_sb = [pha.tile([128, NDT * 128], bf16, tag=f"wq{et}",
                                  name=f"wq{et}") for et in range(2 * HL)]
                wv_sb = [pha.tile([128, HL * HD], bf16, tag=f"wv{dt}",
                                  name=f"wv{dt}") for dt in range(NDT)]
                xt_sb = [pha.tile([128, S], bf16, tag=f"xt{dt}", name=f"xt{dt}")
                         for dt in range(NDT)]
                c2_sb = pha.tile([128, S], f32, tag="c2", name="c2")
                s2_sb = pha.tile([128, S], f32, tag="s2", name="s2")

                def dma_xt_chunk(sc):
                    for dt in range(NDT):
                        nc.sync.dma_start(
                            out=xt_sb[dt][:, sc * 512:(sc + 1) * 512],
                            in_=xt_ext[dt * 128:(dt + 1) * 128,
                                       sc * 512:(sc + 1) * 512])

                def dma_tab_chunk(sc):
                    cl = slice(sc * 512, (sc + 1) * 512)
                    nc.sync.dma_start(out=c2_sb[:, cl], in_=c2_ext[:, cl])
                    nc.sync.dma_start(out=s2_sb[:, cl], in_=s2_ext[:, cl])

                # DMA issue order tuned so compute starts after ~2.5 MB
                nc.sync.dma_start(out=wq_sb[0][:], in_=wqk_ext[0:128, :])
                dma_xt_chunk(0)
                dma_tab_chunk(0)
                for et in range(1, 2 * HL):
                    nc.sync.dma_start(out=wq_sb[et][:],
                                      in_=wqk_ext[et * 128:(et + 1) * 128, :])
                for dt in range(NDT):
                    nc.sync.dma_start(out=wv_sb[dt][:],
                                      in_=wv_ext[dt * 128:(dt + 1) * 128, :])
                for sc in range(1, NSC):
                    dma_xt_chunk(sc)
                    dma_tab_chunk(sc)

                for sc in range(NSC):
                    cl = slice(sc * 512, (sc + 1) * 512)
                    for et in range(2 * HL):
                        ps = psA.tile([128, 512], f32, tag="psA",
                                      name=f"psA_{sc}_{et}")
                        for dt in range(NDT):
                            nc.tensor.matmul(
                                ps[:], wq_sb[et][:, dt * 128:(dt + 1) * 128],
                                xt_sb[dt][:, cl],
                                start=(dt == 0), stop=(dt == NDT - 1))
                        # u = [r*c; i*c]; w = [-i*s; r*s] (s2n = [-sin; sin],
                        # cross-partition reads stay on the PSUM operand);
                        # qk = u + w = [r*c - i*s; i*c + r*s]
                        u = ropep.tile([128, 512], f32, tag="t1",
                                       name=f"t1_{sc}_{et}")
                        w = ropep.tile([128, 512], f32, tag="t2",
                                       name=f"t2_{sc}_{et}")
                        nc.vector.tensor_tensor(out=u[:], in0=ps[:],
                                                in1=c2_sb[:, cl], op=OP.mult)
                        nc.vector.tensor_tensor(out=w[0:64, :],
                                                in0=ps[64:128, :],
                                                in1=s2_sb[0:64, cl],
                                                op=OP.mult)
                        nc.vector.tensor_tensor(out=w[64:128, :],
                                                in0=ps[0:64, :],
                                                in1=s2_sb[64:128, cl],
                                                op=OP.mult)
                        nc.vector.tensor_tensor(out=qk_bf[et][:, cl],
                                                in0=u[:], in1=w[:], op=OP.add)
                    for stl in range(4):
                        st = sc * 4 + stl
                        psv = psA.tile([128, 512], f32, tag="psA",
                                       name=f"psV_{st}")
                        for dt in range(NDT):
                            nc.tensor.matmul(
                                psv[:], xt_sb[dt][:, st * 128:(st + 1) * 128],
                                wv_sb[dt][:],
                                start=(dt == 0), stop=(dt == NDT - 1))
                        nc.scalar.copy(v_bf[st][:], psv[:])

            # -------- Phase B: attention + AllToAll + local out-proj --------
            with tc.tile_pool(name="phb", bufs=1) as phb, \
                 tc.tile_pool(name="att", bufs=4) as attp, \
                 tc.tile_pool(name="psS", bufs=2, space="PSUM") as psS, \
                 tc.tile_pool(name="psPV", bufs=2, space="PSUM") as psPV, \
                 tc.tile_pool(name="psD", bufs=2, space="PSUM") as psD, \
                 tc.tile_pool(name="psPR", bufs=2, space="PSUM") as psPR:
                maskT_sb = [phb.tile([128, 512], f32, tag=f"mk{j}",
                                     name=f"mk{j}") for j in range(4)]
                woT_sb = [phb.tile([128, D], bf16, tag=f"wo{k}", name=f"wo{k}")
                          for k in range(NDT)]
                for j in range(4):
                    nc.sync.dma_start(out=maskT_sb[j][:],
                                      in_=maskT_ext[j * 128:(j + 1) * 128, :])
                for k in range(NDT):
                    nc.sync.dma_start(out=woT_sb[k][:],
                                      in_=woT_ext[k * 128:(k + 1) * 128, :])

                rank = nc.sync.cc_rank(replica_groups=GROUPS8)
                is_b0 = rank < 4
                is_b1 = rank >= 4
                a2a_out = {}

                def attention(qc, hp):
                    qcl = slice(qc * 512, (qc + 1) * 512)
                    nkt = qc * 4 + 4
                    o2p = attp.tile([128, 1024], bf16, tag="o2p",
                                    name=f"o2p_{qc}_{hp}", bufs=3)
                    ps_pv = [psPV.tile([128, 512], f32, tag="pv",
                                       name=f"pv_{qc}_{hp}_{i}")
                             for i in range(2)]
                    ps_d = [psD.tile([1, 512], f32, tag="d",
                                     name=f"d_{qc}_{hp}_{i}") for i in range(2)]
                    for kt in range(nkt):
                        for i in range(2):
                            h = 2 * hp + i
                            ps_s = psS.tile([128, 512], f32, tag="s",
                                            name=f"s_{qc}_{hp}_{kt}_{i}")
                            nc.tensor.matmul(
                                ps_s[:], qk_bf[HL + h][:, kt * 128:(kt + 1) * 128],
                                qk_bf[h][:, qcl], start=True, stop=True)
                            if kt >= qc * 4:
                                nc.vector.tensor_tensor(
                                    out=ps_s[:], in0=ps_s[:],
                                    in1=maskT_sb[kt - qc * 4][:], op=OP.add)
                            e_sb = attp.tile([128, 512], bf16, tag="e",
                                             name=f"e_{qc}_{hp}_{kt}_{i}",
                                             bufs=4)
                            nc.scalar.activation(e_sb[:], ps_s[:], AF.Exp,
                                                 scale=SM_SCALE)
                            nc.tensor.matmul(ps_d[i][:], ones_col[:], e_sb[:],
                                             start=(kt == 0),
                                             stop=(kt == nkt - 1))
                            nc.tensor.matmul(
                                ps_pv[i][:], v_bf[kt][:, h * 128:(h + 1) * 128],
                                e_sb[:], start=(kt == 0), stop=(kt == nkt - 1))
                    for i in range(2):
                        d_sb = attp.tile([1, 512], f32, tag="dsb",
                                         name=f"dsb_{qc}_{hp}_{i}")
                        nc.scalar.copy(d_sb[:], ps_d[i][:])
                        db = attp.tile([128, 512], f32, tag="db",
                                       name=f"db_{qc}_{hp}_{i}", bufs=2)
                        nc.gpsimd.partition_broadcast(db[:], d_sb[:])
                        rb = attp.tile([128, 512], f32, tag="rb",
                                       name=f"rb_{qc}_{hp}_{i}", bufs=2)
                        nc.vector.reciprocal_approx_fast(rb[:], db[:])
                        # o2p column layout is g-major (g = r*2 + i: dest
                        # rank r, pair member i) so the staging DMA is 3-dim
                        nc.vector.tensor_tensor(
                            out=o2p[:].rearrange("p (r i f) -> p i r f",
                                                 i=2, f=128)[:, i],
                            in0=ps_pv[i][:].rearrange("p (r f) -> p r f",
                                                      f=128),
                            in1=rb[:].rearrange("p (r f) -> p r f", f=128),
                            op=OP.mult)
                    # stage O^T blocks to DRAM grouped by destination core
                    # (same block for both batch-groups' slots) and exchange
                    a_in = dram.tile([2048, 128], bf16, tag=f"ain_{qc}_{hp}",
                                     name=f"ain_{qc}_{hp}")
                    src = o2p[:].rearrange("p (g f) -> p g f", f=128)
                    for half in range(2):
                        dst = a_in[half * 1024:(half + 1) * 1024, :].rearrange(
                            "(g p) f -> p g f", p=128)
                        nc.sync.dma_start(out=dst, in_=src)
                    a_out = dram.tile([2048, 128], bf16, tag=f"aout_{qc}_{hp}",
                                      name=f"aout_{qc}_{hp}")
                    nc.gpsimd.collective_compute(
                        "AllToAll", OP.bypass, replica_groups=GROUPS8,
                        ins=[a_in[:].opt()], outs=[a_out[:].opt()])
                    a2a_out[(qc, hp)] = a_out

                def outproj(qc):
                    # lhs block m = hp*8 + r*2 + i holds global ocol block
                    # (head) k = r*4 + 2*hp + i
                    lhs = attp.tile([128, D], bf16, tag="lhs",
                                    name=f"lhs_{qc}", bufs=2)
                    for hp in range(2):
                        a_out = a2a_out[(qc, hp)]
                        dst = lhs[:, hp * 1024:(hp + 1) * 1024].rearrange(
                            "p (g f) -> p g f", f=128)
                        for b, cond in ((0, is_b0), (1, is_b1)):
                            src = a_out[b * 1024:(b + 1) * 1024, :].rearrange(
                                "(g p) f -> p g f", p=128)
                            nc.sync.dma_start(out=dst, in_=src, cond=cond)
                    mks = [(hp * 8 + r * 2 + i, r * HL + 2 * hp + i)
                           for hp in range(2) for r in range(TP)
                           for i in range(2)]
                    for ec in range(4):
                        ps = psPR.tile([128, 512], f32, tag="pr",
                                       name=f"pr_{qc}_{ec}")
                        for n, (m, k) in enumerate(mks):
                            nc.tensor.matmul(
                                ps[:], lhs[:, m * 128:(m + 1) * 128],
                                woT_sb[k][:, ec * 512:(ec + 1) * 512],
                                start=(n == 0), stop=(n == len(mks) - 1))
                        fin = attp.tile([128, 512], bf16, tag="fin",
                                        name=f"fin_{qc}_{ec}", bufs=4)
                        nc.vector.tensor_copy(fin[:], ps[:])
                        nc.sync.dma_start(
                            out=out_ext[qc * 128:(qc + 1) * 128,
                                        ec * 512:(ec + 1) * 512],
                            in_=fin[:])

                prev = None
                for qc in CHUNK_ORDER:
                    attention(qc, 0)
                    if prev is not None:
                        outproj(prev)
                    attention(qc, 1)
                    prev = qc
                outproj(prev)
    nc.finalize()
    return nc


def _prep_inputs(x, freqs_cos, freqs_sin, mask, wqkv, wo):
    bf = ml_dtypes.bfloat16
    perm = np.concatenate([np.arange(0, HD, 2), np.arange(1, HD, 2)])
    mask2d = np.asarray(mask, np.float32).reshape(S, S)
    maskT = np.ascontiguousarray(np.concatenate(
        [np.maximum(mask2d[0:512, j * 128:(j + 1) * 128].T, -1e30)
         for j in range(4)], axis=0)).astype(np.float32)
    cosT = np.asarray(freqs_cos, np.float32).T   # [64, S]
    sinT = np.asarray(freqs_sin, np.float32).T
    c2 = np.ascontiguousarray(np.concatenate([cosT, cosT], axis=0))
    s2 = np.ascontiguousarray(np.concatenate([-sinT, sinT], axis=0))
    wqkv = np.asarray(wqkv, np.float32)
    wo = np.asarray(wo, np.float32)
    x = np.asarray(x, np.float32)
    woT = np.ascontiguousarray(wo.T).astype(bf)   # [2048 o, 2048 e]

    in_maps = []
    for c in range(8):
        b, r = divmod(c, TP)
        heads = list(range(r * HL, (r + 1) * HL))
        # q/k weights: per (sec, head) block in SBUF layout [128 p=d%128,
        # (dt c)=hd], i.e. transpose of blk[c, dt*128+p]
        rows = []
        for sec in range(2):
            for h in heads:
                blk = wqkv[sec * D + h * HD: sec * D + (h + 1) * HD][perm]
                b3 = blk.reshape(HD, NDT, 128)          # [hd, dt, p]
                rows.append(np.transpose(b3, (2, 1, 0)).reshape(128, -1))
        wqk = np.ascontiguousarray(np.concatenate(rows, axis=0)).astype(bf)
        wv = np.ascontiguousarray(np.concatenate(
            [wqkv[2 * D + h * HD: 2 * D + (h + 1) * HD].T for h in heads],
            axis=1)).astype(bf)                          # [2048, 512]
        xt = np.ascontiguousarray(x[b].T).astype(bf)
        in_maps.append({"xt": xt, "wqk": wqk, "wv": wv, "c2": c2, "s2": s2,
                        "maskT": maskT, "woT": woT})
    return in_maps


def kernel(x, freqs_cos, freqs_sin, mask, wqkv, wo, input_pos=None,
           _want_res=False, _trace=False, _tmpdir=None):
    from concourse.bass_utils import run_bass_kernel_spmd

    if "nc" not in _cache:
        _cache["nc"] = _build_graph()
    nc = _cache["nc"]

    in_maps = _prep_inputs(x, freqs_cos, freqs_sin, mask, wqkv, wo)
    kw = {}
    if _trace:
        kw = dict(trace=True, tmpdir=_tmpdir)
    res = run_bass_kernel_spmd(nc, in_maps, list(range(8)), **kw)

    y = np.empty((B, S, D), np.float32)
    for c in range(8):
        b, r = divmod(c, TP)
        oc = np.asarray(res.results[c]["out"], np.float32)
        for qc in range(NQC):
            qt = 4 * qc + r
            y[b, qt * 128:(qt + 1) * 128, :] = oc[qc * 128:(qc + 1) * 128]
    if _want_res:
        return y, res
    return y
